# revision 42
# baseline (speedup 1.0000x reference)
"""Trainium2 Bass kernel for CustomLSTMForecast.

B=512, T=256, I=256, H=512. Data-parallel: batch sharded 8 ways (64
rows/core), LSTM + fc weights replicated.

Per-core design (batch m = 64), v2:

Cell-state layout ("c-layout") [128, 256]: partition p = b + 64*ph
(b = batch row, ph = hidden half), column j = hidden unit u - 256*ph.
Every elementwise op uses all 128 partitions.

Gates land in two PSUM tiles per step, four 256-col strips:
  pA [128,512]: cols 0:256 = f, 256:512 = i     (c-layout per strip)
  pB [128,512]: cols 0:256 = c_hat, 256:512 = o
Each strip is accumulated by N=256 matmuls: bias (K=1 ones row,
start) + 2 x-chunks + 4 h-chunks (stop on k=3). The two partition
halves (ph) of a strip pair up in PE column groups (auto
tile_position from out.base_partition in {0, 64}), so two M=64
matmuls run concurrently.

h is produced as two bf16 [128,128] tiles (h_a = hidden cols 0:128 of
each half, h_b = 128:256), PE-transposed quadrant-wise into
hTa (chunks 0,2) / hTb (chunks 1,3), so next step's h-matmuls start
after the first copy (k-waves {0,2} then {1,3}).

x-part + bias matmuls for step t+1 are emitted before step t's
elementwise so the PE never idles long enough to re-throttle (HAM).
"""
from contextlib import ExitStack

import numpy as np

import concourse.bass as bass
import concourse.tile as tile
from concourse import bacc, mybir
from concourse.bass_utils import run_bass_kernel_spmd

F32 = mybir.dt.float32
BF16 = mybir.dt.bfloat16
AF = mybir.ActivationFunctionType
ALU = mybir.AluOpType

B, T, I, H = 512, 256, 256, 512
NCORES = 8
BC = B // NCORES          # 64 batch rows per core
KH = H // 128             # 4 hidden k-chunks
KX = I // 128             # 2 input k-chunks
NK = KH + KX              # 6 contraction chunks (no bias chunk)

# gate order in the repacked W: g=0 f, 1 i, 2 c_hat, 3 o.
# W_w row blocks are (f, i, o, c_hat) -> col-reorder [0:1024, 1536:2048,
# 1024:1536].
SORD = (0, 1, 2, 3)       # strip completion / emission order: f, i, c, o

_CACHE = {}


def _build(nsteps=T):
    if nsteps in _CACHE:
        return _CACHE[nsteps]
    nc = bacc.Bacc("TRN2", target_bir_lowering=False, debug=False,
                   num_devices=NCORES)
    d_x = nc.dram_tensor("xT", [nsteps, 128, KX, BC], BF16,
                         kind="ExternalInput").ap()
    d_w = nc.dram_tensor("W", [NK, 128, 4 * H], BF16,
                         kind="ExternalInput").ap()
    d_wb = nc.dram_tensor("Wb", [1, 2, 1024], BF16,
                          kind="ExternalInput").ap()
    d_ones = nc.dram_tensor("ones_row", [1, BC], BF16,
                            kind="ExternalInput").ap()
    d_eye2 = nc.dram_tensor("eye2", [BC, BC], BF16,
                            kind="ExternalInput").ap()
    d_fcw = nc.dram_tensor("fcw", [BC, 512], F32, kind="ExternalInput").ap()
    d_out = nc.dram_tensor("out", [BC, 1], F32, kind="ExternalOutput").ap()

    with tile.TileContext(nc) as tc, ExitStack() as ctx:
        _body(tc, ctx, nsteps, d_x, d_w, d_wb, d_ones, d_eye2, d_fcw, d_out)
    nc.compile()
    _CACHE[nsteps] = nc
    return nc


def _body(tc, ctx, nsteps, d_x, d_w, d_wb, d_ones, d_eye2, d_fcw, d_out):
    nc = tc.nc
    const = ctx.enter_context(tc.tile_pool(name="const", bufs=1))
    xpool = ctx.enter_context(tc.tile_pool(name="x", bufs=4))
    gact = ctx.enter_context(tc.tile_pool(name="gact", bufs=2))
    state = ctx.enter_context(tc.tile_pool(name="state", bufs=2))
    # Per-strip PSUM tiles: Tile's dependency tracking is per-tile, so
    # separate tiles let each activation start as soon as its own
    # strip's accumulation finishes. {f,i} share one bank-wide tile
    # (both sigmoid -> a single [128,512] ACT op). Single-buffered ON
    # PURPOSE: the WAR dep (next step's bias waits on this step's
    # activation read) paces the PE across the elementwise window,
    # which keeps HAM from re-throttling the clock (bufs=2 measured
    # 555us of cold-clock vs 69us).
    psFI = ctx.enter_context(tc.tile_pool(name="psFI", bufs=1, space="PSUM"))
    psC = ctx.enter_context(tc.tile_pool(name="psC", bufs=1, space="PSUM"))
    psO = ctx.enter_context(tc.tile_pool(name="psO", bufs=1, space="PSUM"))
    psTa = ctx.enter_context(tc.tile_pool(name="psTa", bufs=1, space="PSUM"))
    psTb = ctx.enter_context(tc.tile_pool(name="psTb", bufs=1, space="PSUM"))

    sW = const.tile([128, NK, 4 * H], BF16)
    nc.sync.dma_start(out=sW[:], in_=d_w.rearrange("k p n -> p k n"))
    sWb = const.tile([1, 2, 1024], BF16)
    nc.sync.dma_start(out=sWb[:], in_=d_wb)
    s_ones = const.tile([1, BC], BF16)
    nc.sync.dma_start(out=s_ones[:], in_=d_ones)
    s_eye2 = const.tile([BC, BC], BF16)
    nc.sync.dma_start(out=s_eye2[:], in_=d_eye2)
    s_fcw = const.tile([BC, 512], F32)
    nc.sync.dma_start(out=s_fcw[:], in_=d_fcw)

    c_prev = state.tile([128, 256], F32, tag="c")
    nc.vector.memset(c_prev[:], 0.0)

    def gate_out(strips, g, ph):
        # gate g -> (tile, col offset): f,i share the FI tile
        fi, c, o = strips
        ps, co = (fi, 256 * g) if g < 2 else (c, 0) if g == 2 else (o, 0)
        return ps[64 * ph:64 * ph + 64, co:co + 256]

    def emit_bias_x(t, strips, stop_after_x):
        xs = xpool.tile([128, KX, BC], BF16, tag="xs")
        nc.sync.dma_start(out=xs[:], in_=d_x[t])
        fi, c, o = strips
        # bias first (paced by the previous step's activation reads via
        # WAR on the single-buffered banks), then x g-major.
        for ph in range(2):
            nc.tensor.matmul(fi[64 * ph:64 * ph + 64, :], s_ones[:],
                             sWb[:, ph, 0:512], start=True, stop=False)
        for ph in range(2):
            nc.tensor.matmul(c[64 * ph:64 * ph + 64, :], s_ones[:],
                             sWb[:, ph, 512:768], start=True, stop=False)
        for ph in range(2):
            nc.tensor.matmul(o[64 * ph:64 * ph + 64, :], s_ones[:],
                             sWb[:, ph, 768:1024], start=True, stop=False)
        for g in SORD:
            for kx in range(KX):
                for ph in range(2):
                    rhs = sW[:, KH + kx, 512 * g + 256 * ph:
                             512 * g + 256 * ph + 256]
                    nc.tensor.matmul(gate_out(strips, g, ph),
                                     xs[:, kx, :], rhs, start=False,
                                     stop=(stop_after_x and kx == KX - 1))

    def emit_h_wave(strips, wave, hT):
        # wave: (0, 1) with hTa, or (2, 3) with hTb
        for g in SORD:
            for wi, k in enumerate(wave):
                for ph in range(2):
                    rhs = sW[:, k, 512 * g + 256 * ph:
                             512 * g + 256 * ph + 256]
                    nc.tensor.matmul(gate_out(strips, g, ph),
                                     hT[:, wi, :], rhs,
                                     start=False, stop=(k == 3))

    def new_strips():
        fi = psFI.tile([128, 512], F32, tag="FI")
        c = psC.tile([128, 256], F32, tag="C")
        o = psO.tile([128, 256], F32, tag="O")
        return (fi, c, o)

    # prologue: gates(0) = bias + x only
    strips = new_strips()
    emit_bias_x(0, strips, stop_after_x=True)

    h_lo = h_hi = None
    for t in range(nsteps):
        last = t == nsteps - 1
        if not last:
            strips_n = new_strips()
            emit_bias_x(t + 1, strips_n, stop_after_x=False)

        # elementwise for step t; strips = (fi, c, o)
        fi_t, c_t, o_t = strips
        sfi = gact.tile([128, 512], F32, tag="sfi")
        nc.scalar.activation(sfi[:], fi_t[:], AF.Sigmoid)
        tc_ = gact.tile([128, 256], F32, tag="tc")
        nc.scalar.activation(tc_[:], c_t[:], AF.Tanh)
        so = gact.tile([128, 256], BF16, tag="so")
        nc.scalar.activation(so[:], o_t[:], AF.Sigmoid)

        u1 = gact.tile([128, 256], F32, tag="u1")
        nc.vector.tensor_mul(u1[:], c_prev[:], sfi[:, 0:256])
        u2 = gact.tile([128, 256], F32, tag="u2")
        nc.vector.tensor_mul(u2[:], sfi[:, 256:512], tc_[:])
        c_new = state.tile([128, 256], F32, tag="c")
        nc.vector.tensor_add(c_new[:], u1[:], u2[:])
        tch = gact.tile([128, 256], BF16, tag="tch")
        nc.scalar.activation(tch[:], c_new[:], AF.Tanh)
        # h split by hidden half, both landed at partition base 0
        # (engines support partition-shifted reads) so the PE transposes
        # only see base-0 stationaries (base-64 ones crash the PE).
        h_lo = state.tile([BC, 256], BF16, tag="hlo")    # hid 0:256
        nc.vector.tensor_mul(h_lo[:], so[0:64, :], tch[0:64, :])
        c_prev = c_new

        if not last:
            # transpose+copy chunks (0,1) from h_lo BEFORE the h_hi mul
            # so copy_a isn't queued behind it on the DVE and wave (0,1)
            # starts earlier.
            pTa = psTa.tile([128, 2, BC], BF16, tag="pTa")
            nc.tensor.transpose(pTa[:, 0, :], h_lo[:, 0:128], s_eye2[:])
            nc.tensor.transpose(pTa[:, 1, :], h_lo[:, 128:256], s_eye2[:])
            hTa = state.tile([128, 2, BC], BF16, tag="hTa")
            nc.vector.tensor_copy(hTa[:], pTa[:])
            h_hi = state.tile([BC, 256], BF16, tag="hhi")    # hid 256:512
            nc.vector.tensor_mul(h_hi[:], so[64:128, :], tch[64:128, :])
            pTb = psTb.tile([128, 2, BC], BF16, tag="pTb")
            nc.tensor.transpose(pTb[:, 0, :], h_hi[:, 0:128], s_eye2[:])
            nc.tensor.transpose(pTb[:, 1, :], h_hi[:, 128:256], s_eye2[:])
            hTb = state.tile([128, 2, BC], BF16, tag="hTb")
            nc.vector.tensor_copy(hTb[:], pTb[:])
            emit_h_wave(strips_n, (0, 1), hTa)
            emit_h_wave(strips_n, (2, 3), hTb)
            strips = strips_n
        else:
            h_hi = state.tile([BC, 256], BF16, tag="hhi")
            nc.vector.tensor_mul(h_hi[:], so[64:128, :], tch[64:128, :])

    # fc head: out[b] = sum_u h[b, u] * fc_w[u]; host adds fc_b
    ra = gact.tile([BC, 1], F32, tag="ra")
    ma = gact.tile([BC, 256], F32, tag="ma")
    nc.vector.scalar_tensor_tensor(ma[:], h_lo[:], 1.0, s_fcw[:, 0:256],
                                   op0=ALU.mult, op1=ALU.mult,
                                   accum_out=ra[:])
    rb = gact.tile([BC, 1], F32, tag="rb")
    mb = gact.tile([BC, 256], F32, tag="mb")
    nc.vector.scalar_tensor_tensor(mb[:], h_hi[:], 1.0, s_fcw[:, 256:512],
                                   op0=ALU.mult, op1=ALU.mult,
                                   accum_out=rb[:])
    ro = gact.tile([BC, 1], F32, tag="ro")
    nc.vector.tensor_add(ro[:], ra[:], rb[:])
    nc.sync.dma_start(out=d_out, in_=ro[:])


def _prep_core_inputs(x, W_w, W_b, fc_w, fc_b, core, nsteps=T):
    """Host-side shard + relayout for one core."""
    xs = x[core * BC:(core + 1) * BC, :nsteps]          # [BC, t, I]
    xt = np.ascontiguousarray(xs.transpose(1, 2, 0))    # [t, I, BC]
    xt = xt.reshape(nsteps, KX, 128, BC).transpose(0, 2, 1, 3)
    xt = np.ascontiguousarray(xt)                       # [t, 128, KX, BC]

    # gate reorder (f, i, o, c_hat) -> (f, i, c_hat, o)
    perm = np.concatenate([np.arange(0, 1024), np.arange(1536, 2048),
                           np.arange(1024, 1536)])
    WT = W_w.T[:, perm]                                 # [768, 2048]
    wt = np.ascontiguousarray(WT.reshape(NK, 128, 4 * H))
    # bias cols per ph: [f 256 | i 256 | c 256 | o 256] -> [1, 2, 1024]
    wb_re = np.ascontiguousarray(
        W_b[perm].reshape(4, 2, 256).transpose(1, 0, 2).reshape(1, 2, 1024))

    ones_row = np.ones((1, BC), dtype=np.float32)
    eye2 = np.eye(BC, dtype=np.float32)
    fcw = np.ascontiguousarray(
        np.broadcast_to(fc_w.reshape(1, H), (BC, H)).astype(np.float32))

    import ml_dtypes
    bf = ml_dtypes.bfloat16
    return {"xT": xt.astype(bf), "W": wt.astype(bf),
            "Wb": wb_re.astype(bf), "ones_row": ones_row.astype(bf),
            "eye2": eye2.astype(bf), "fcw": fcw}


def kernel(x, W_w, W_b, fc_w, fc_b):
    x = np.asarray(x, dtype=np.float32)
    W_w = np.asarray(W_w, dtype=np.float32)
    W_b = np.asarray(W_b, dtype=np.float32)
    fc_w = np.asarray(fc_w, dtype=np.float32)
    fc_b = np.asarray(fc_b, dtype=np.float32)

    nc = _build(T)
    in_maps = [_prep_core_inputs(x, W_w, W_b, fc_w, fc_b, c)
               for c in range(NCORES)]
    res = run_bass_kernel_spmd(nc, in_maps, list(range(NCORES))).results
    return np.concatenate(
        [res[c]["out"] + np.float32(fc_b[0]) for c in range(NCORES)], axis=0)


# revision 44
# speedup vs baseline: 1.0815x; 1.0815x over previous
"""Trainium2 Bass kernel for CustomLSTMForecast.

B=512, T=256, I=256, H=512. Data-parallel: batch sharded 8 ways (64
rows/core), LSTM + fc weights replicated.

Per-core design (batch m = 64), v2:

Cell-state layout ("c-layout") [128, 256]: partition p = b + 64*ph
(b = batch row, ph = hidden half), column j = hidden unit u - 256*ph.
Every elementwise op uses all 128 partitions.

Gates land in two PSUM tiles per step, four 256-col strips:
  pA [128,512]: cols 0:256 = f, 256:512 = i     (c-layout per strip)
  pB [128,512]: cols 0:256 = c_hat, 256:512 = o
Each strip is accumulated by N=256 matmuls: bias (K=1 ones row,
start) + 2 x-chunks + 4 h-chunks (stop on k=3). The two partition
halves (ph) of a strip pair up in PE column groups (auto
tile_position from out.base_partition in {0, 64}), so two M=64
matmuls run concurrently.

h is produced as two bf16 [128,128] tiles (h_a = hidden cols 0:128 of
each half, h_b = 128:256), PE-transposed quadrant-wise into
hTa (chunks 0,2) / hTb (chunks 1,3), so next step's h-matmuls start
after the first copy (k-waves {0,2} then {1,3}).

x-part + bias matmuls for step t+1 are emitted before step t's
elementwise so the PE never idles long enough to re-throttle (HAM).
"""
from contextlib import ExitStack

import numpy as np

import concourse.bass as bass
import concourse.tile as tile
from concourse import bacc, mybir
from concourse.bass_utils import run_bass_kernel_spmd

F32 = mybir.dt.float32
BF16 = mybir.dt.bfloat16
AF = mybir.ActivationFunctionType
ALU = mybir.AluOpType

B, T, I, H = 512, 256, 256, 512
NCORES = 8
BC = B // NCORES          # 64 batch rows per core
KH = H // 128             # 4 hidden k-chunks
KX = I // 128             # 2 input k-chunks
NK = KH + KX              # 6 contraction chunks (no bias chunk)

# gate order in the repacked W: g=0 f, 1 i, 2 c_hat, 3 o.
# W_w row blocks are (f, i, o, c_hat) -> col-reorder [0:1024, 1536:2048,
# 1024:1536].
SORD = (0, 1, 2, 3)       # strip completion / emission order: f, i, c, o

_CACHE = {}


def _build(nsteps=T):
    if nsteps in _CACHE:
        return _CACHE[nsteps]
    nc = bacc.Bacc("TRN2", target_bir_lowering=False, debug=False,
                   num_devices=NCORES)
    d_x = nc.dram_tensor("xT", [nsteps, 128, KX, BC], BF16,
                         kind="ExternalInput").ap()
    d_w = nc.dram_tensor("W", [NK, 128, 4 * H], BF16,
                         kind="ExternalInput").ap()
    d_wb = nc.dram_tensor("Wb", [1, 2, 1024], BF16,
                          kind="ExternalInput").ap()
    d_ones = nc.dram_tensor("ones_row", [1, BC], BF16,
                            kind="ExternalInput").ap()
    d_eye2 = nc.dram_tensor("eye2", [BC, BC], BF16,
                            kind="ExternalInput").ap()
    d_fcw = nc.dram_tensor("fcw", [BC, 512], F32, kind="ExternalInput").ap()
    d_out = nc.dram_tensor("out", [BC, 1], F32, kind="ExternalOutput").ap()

    with tile.TileContext(nc) as tc, ExitStack() as ctx:
        _body(tc, ctx, nsteps, d_x, d_w, d_wb, d_ones, d_eye2, d_fcw, d_out)
    nc.compile()
    _CACHE[nsteps] = nc
    return nc


def _body(tc, ctx, nsteps, d_x, d_w, d_wb, d_ones, d_eye2, d_fcw, d_out):
    nc = tc.nc
    const = ctx.enter_context(tc.tile_pool(name="const", bufs=1))
    xpool = ctx.enter_context(tc.tile_pool(name="x", bufs=4))
    gact = ctx.enter_context(tc.tile_pool(name="gact", bufs=2))
    state = ctx.enter_context(tc.tile_pool(name="state", bufs=2))
    # Per-strip PSUM tiles: Tile's dependency tracking is per-tile, so
    # separate tiles let each activation start as soon as its own
    # strip's accumulation finishes. {f,i} share one bank-wide tile
    # (both sigmoid -> a single [128,512] ACT op). Single-buffered ON
    # PURPOSE: the WAR dep (next step's bias waits on this step's
    # activation read) paces the PE across the elementwise window,
    # which keeps HAM from re-throttling the clock (bufs=2 measured
    # 555us of cold-clock vs 69us).
    psFI = ctx.enter_context(tc.tile_pool(name="psFI", bufs=1, space="PSUM"))
    psC = ctx.enter_context(tc.tile_pool(name="psC", bufs=1, space="PSUM"))
    psO = ctx.enter_context(tc.tile_pool(name="psO", bufs=1, space="PSUM"))
    psTa = ctx.enter_context(tc.tile_pool(name="psTa", bufs=1, space="PSUM"))
    psTb = ctx.enter_context(tc.tile_pool(name="psTb", bufs=1, space="PSUM"))

    sW = const.tile([128, NK, 4 * H], BF16)
    nc.sync.dma_start(out=sW[:], in_=d_w.rearrange("k p n -> p k n"))
    sWb = const.tile([1, 2, 1024], BF16)
    nc.sync.dma_start(out=sWb[:], in_=d_wb)
    s_ones = const.tile([1, BC], BF16)
    nc.sync.dma_start(out=s_ones[:], in_=d_ones)
    s_eye2 = const.tile([BC, BC], BF16)
    nc.sync.dma_start(out=s_eye2[:], in_=d_eye2)
    s_fcw = const.tile([BC, 512], F32)
    nc.sync.dma_start(out=s_fcw[:], in_=d_fcw)

    c_prev = state.tile([128, 256], F32, tag="c")
    nc.vector.memset(c_prev[:], 0.0)

    def gate_out(strips, g, ph):
        # gate g -> (tile, col offset): f,i share the FI tile
        fi, c, o = strips
        ps, co = (fi, 256 * g) if g < 2 else (c, 0) if g == 2 else (o, 0)
        return ps[64 * ph:64 * ph + 64, co:co + 256]

    def emit_bias_x(t, strips, stop_after_x):
        xs = xpool.tile([128, KX, BC], BF16, tag="xs")
        nc.sync.dma_start(out=xs[:], in_=d_x[t])
        fi, c, o = strips
        # bias first (paced by the previous step's activation reads via
        # WAR on the single-buffered banks), then x g-major.
        for ph in range(2):
            nc.tensor.matmul(fi[64 * ph:64 * ph + 64, :], s_ones[:],
                             sWb[:, ph, 0:512], start=True, stop=False)
        for ph in range(2):
            nc.tensor.matmul(c[64 * ph:64 * ph + 64, :], s_ones[:],
                             sWb[:, ph, 512:768], start=True, stop=False)
        for ph in range(2):
            nc.tensor.matmul(o[64 * ph:64 * ph + 64, :], s_ones[:],
                             sWb[:, ph, 768:1024], start=True, stop=False)
        for g in SORD:
            for kx in range(KX):
                for ph in range(2):
                    rhs = sW[:, KH + kx, 512 * g + 256 * ph:
                             512 * g + 256 * ph + 256]
                    nc.tensor.matmul(gate_out(strips, g, ph),
                                     xs[:, kx, :], rhs, start=False,
                                     stop=(stop_after_x and kx == KX - 1))

    def emit_h_all(strips, hTa, hTb):
        # strip-major over ALL k-chunks so each gate tile completes as
        # early as possible (fi after 8 pairs, not 12): k 0,1 from hTa
        # (h_lo), k 2,3 from hTb (h_hi).
        for g in SORD:
            for k in range(KH):
                hT = hTa if k < 2 else hTb
                for ph in range(2):
                    rhs = sW[:, k, 512 * g + 256 * ph:
                             512 * g + 256 * ph + 256]
                    nc.tensor.matmul(gate_out(strips, g, ph),
                                     hT[:, k % 2, :], rhs,
                                     start=False, stop=(k == 3))

    def new_strips():
        fi = psFI.tile([128, 512], F32, tag="FI")
        c = psC.tile([128, 256], F32, tag="C")
        o = psO.tile([128, 256], F32, tag="O")
        return (fi, c, o)

    # prologue: gates(0) = bias + x only
    strips = new_strips()
    emit_bias_x(0, strips, stop_after_x=True)

    h_lo = h_hi = None
    for t in range(nsteps):
        last = t == nsteps - 1
        if not last:
            strips_n = new_strips()
            emit_bias_x(t + 1, strips_n, stop_after_x=False)

        # elementwise for step t; strips = (fi, c, o)
        fi_t, c_t, o_t = strips
        sfi = gact.tile([128, 512], F32, tag="sfi")
        nc.scalar.activation(sfi[:], fi_t[:], AF.Sigmoid)
        tc_ = gact.tile([128, 256], F32, tag="tc")
        nc.scalar.activation(tc_[:], c_t[:], AF.Tanh)
        so = gact.tile([128, 256], BF16, tag="so")
        nc.scalar.activation(so[:], o_t[:], AF.Sigmoid)

        u1 = gact.tile([128, 256], F32, tag="u1")
        nc.vector.tensor_mul(u1[:], c_prev[:], sfi[:, 0:256])
        u2 = gact.tile([128, 256], F32, tag="u2")
        nc.vector.tensor_mul(u2[:], sfi[:, 256:512], tc_[:])
        c_new = state.tile([128, 256], F32, tag="c")
        nc.vector.tensor_add(c_new[:], u1[:], u2[:])
        tch = gact.tile([128, 256], BF16, tag="tch")
        nc.scalar.activation(tch[:], c_new[:], AF.Tanh)
        # h split by hidden half, both landed at partition base 0
        # (engines support partition-shifted reads) so the PE transposes
        # only see base-0 stationaries (base-64 ones crash the PE).
        h_lo = state.tile([BC, 256], BF16, tag="hlo")    # hid 0:256
        nc.vector.tensor_mul(h_lo[:], so[0:64, :], tch[0:64, :])
        c_prev = c_new

        if not last:
            # transpose+copy chunks (0,1) from h_lo BEFORE the h_hi mul
            # so copy_a isn't queued behind it on the DVE and wave (0,1)
            # starts earlier.
            pTa = psTa.tile([128, 2, BC], BF16, tag="pTa")
            nc.tensor.transpose(pTa[:, 0, :], h_lo[:, 0:128], s_eye2[:])
            nc.tensor.transpose(pTa[:, 1, :], h_lo[:, 128:256], s_eye2[:])
            hTa = state.tile([128, 2, BC], BF16, tag="hTa")
            nc.vector.tensor_copy(hTa[:], pTa[:])
            h_hi = state.tile([BC, 256], BF16, tag="hhi")    # hid 256:512
            nc.vector.tensor_mul(h_hi[:], so[64:128, :], tch[64:128, :])
            pTb = psTb.tile([128, 2, BC], BF16, tag="pTb")
            nc.tensor.transpose(pTb[:, 0, :], h_hi[:, 0:128], s_eye2[:])
            nc.tensor.transpose(pTb[:, 1, :], h_hi[:, 128:256], s_eye2[:])
            hTb = state.tile([128, 2, BC], BF16, tag="hTb")
            nc.vector.tensor_copy(hTb[:], pTb[:])
            emit_h_all(strips_n, hTa, hTb)
            strips = strips_n
        else:
            h_hi = state.tile([BC, 256], BF16, tag="hhi")
            nc.vector.tensor_mul(h_hi[:], so[64:128, :], tch[64:128, :])

    # fc head: out[b] = sum_u h[b, u] * fc_w[u]; host adds fc_b
    ra = gact.tile([BC, 1], F32, tag="ra")
    ma = gact.tile([BC, 256], F32, tag="ma")
    nc.vector.scalar_tensor_tensor(ma[:], h_lo[:], 1.0, s_fcw[:, 0:256],
                                   op0=ALU.mult, op1=ALU.mult,
                                   accum_out=ra[:])
    rb = gact.tile([BC, 1], F32, tag="rb")
    mb = gact.tile([BC, 256], F32, tag="mb")
    nc.vector.scalar_tensor_tensor(mb[:], h_hi[:], 1.0, s_fcw[:, 256:512],
                                   op0=ALU.mult, op1=ALU.mult,
                                   accum_out=rb[:])
    ro = gact.tile([BC, 1], F32, tag="ro")
    nc.vector.tensor_add(ro[:], ra[:], rb[:])
    nc.sync.dma_start(out=d_out, in_=ro[:])


def _prep_core_inputs(x, W_w, W_b, fc_w, fc_b, core, nsteps=T):
    """Host-side shard + relayout for one core."""
    xs = x[core * BC:(core + 1) * BC, :nsteps]          # [BC, t, I]
    xt = np.ascontiguousarray(xs.transpose(1, 2, 0))    # [t, I, BC]
    xt = xt.reshape(nsteps, KX, 128, BC).transpose(0, 2, 1, 3)
    xt = np.ascontiguousarray(xt)                       # [t, 128, KX, BC]

    # gate reorder (f, i, o, c_hat) -> (f, i, c_hat, o)
    perm = np.concatenate([np.arange(0, 1024), np.arange(1536, 2048),
                           np.arange(1024, 1536)])
    WT = W_w.T[:, perm]                                 # [768, 2048]
    wt = np.ascontiguousarray(WT.reshape(NK, 128, 4 * H))
    # bias cols per ph: [f 256 | i 256 | c 256 | o 256] -> [1, 2, 1024]
    wb_re = np.ascontiguousarray(
        W_b[perm].reshape(4, 2, 256).transpose(1, 0, 2).reshape(1, 2, 1024))

    ones_row = np.ones((1, BC), dtype=np.float32)
    eye2 = np.eye(BC, dtype=np.float32)
    fcw = np.ascontiguousarray(
        np.broadcast_to(fc_w.reshape(1, H), (BC, H)).astype(np.float32))

    import ml_dtypes
    bf = ml_dtypes.bfloat16
    return {"xT": xt.astype(bf), "W": wt.astype(bf),
            "Wb": wb_re.astype(bf), "ones_row": ones_row.astype(bf),
            "eye2": eye2.astype(bf), "fcw": fcw}


def kernel(x, W_w, W_b, fc_w, fc_b):
    x = np.asarray(x, dtype=np.float32)
    W_w = np.asarray(W_w, dtype=np.float32)
    W_b = np.asarray(W_b, dtype=np.float32)
    fc_w = np.asarray(fc_w, dtype=np.float32)
    fc_b = np.asarray(fc_b, dtype=np.float32)

    nc = _build(T)
    in_maps = [_prep_core_inputs(x, W_w, W_b, fc_w, fc_b, c)
               for c in range(NCORES)]
    res = run_bass_kernel_spmd(nc, in_maps, list(range(NCORES))).results
    return np.concatenate(
        [res[c]["out"] + np.float32(fc_b[0]) for c in range(NCORES)], axis=0)


# revision 49
# speedup vs baseline: 1.1032x; 1.0201x over previous
"""Trainium2 Bass kernel for CustomLSTMForecast.

B=512, T=256, I=256, H=512. Data-parallel: batch sharded 8 ways (64
rows/core), LSTM + fc weights replicated.

Per-core design (batch m = 64), v2:

Cell-state layout ("c-layout") [128, 256]: partition p = b + 64*ph
(b = batch row, ph = hidden half), column j = hidden unit u - 256*ph.
Every elementwise op uses all 128 partitions.

Gates land in two PSUM tiles per step, four 256-col strips:
  pA [128,512]: cols 0:256 = f, 256:512 = i     (c-layout per strip)
  pB [128,512]: cols 0:256 = c_hat, 256:512 = o
Each strip is accumulated by N=256 matmuls: bias (K=1 ones row,
start) + 2 x-chunks + 4 h-chunks (stop on k=3). The two partition
halves (ph) of a strip pair up in PE column groups (auto
tile_position from out.base_partition in {0, 64}), so two M=64
matmuls run concurrently.

h is produced as two bf16 [128,128] tiles (h_a = hidden cols 0:128 of
each half, h_b = 128:256), PE-transposed quadrant-wise into
hTa (chunks 0,2) / hTb (chunks 1,3), so next step's h-matmuls start
after the first copy (k-waves {0,2} then {1,3}).

x-part + bias matmuls for step t+1 are emitted before step t's
elementwise so the PE never idles long enough to re-throttle (HAM).
"""
from contextlib import ExitStack

import numpy as np

import concourse.bass as bass
import concourse.tile as tile
from concourse import bacc, mybir
from concourse.bass_utils import run_bass_kernel_spmd

F32 = mybir.dt.float32
BF16 = mybir.dt.bfloat16
AF = mybir.ActivationFunctionType
ALU = mybir.AluOpType

B, T, I, H = 512, 256, 256, 512
NCORES = 8
BC = B // NCORES          # 64 batch rows per core
KH = H // 128             # 4 hidden k-chunks
KX = I // 128             # 2 input k-chunks
NK = KH + KX              # 6 contraction chunks (no bias chunk)

# gate order in the repacked W: g=0 f, 1 i, 2 c_hat, 3 o.
# W_w row blocks are (f, i, o, c_hat) -> col-reorder [0:1024, 1536:2048,
# 1024:1536].
SORD = (0, 1, 2, 3)       # strip completion / emission order: f, i, c, o

_CACHE = {}


def _build(nsteps=T):
    if nsteps in _CACHE:
        return _CACHE[nsteps]
    nc = bacc.Bacc("TRN2", target_bir_lowering=False, debug=False,
                   num_devices=NCORES)
    d_x = nc.dram_tensor("xT", [nsteps, 128, KX, BC], BF16,
                         kind="ExternalInput").ap()
    d_w = nc.dram_tensor("W", [NK, 128, 4 * H], BF16,
                         kind="ExternalInput").ap()
    d_wb = nc.dram_tensor("Wb", [1, 2, 1024], BF16,
                          kind="ExternalInput").ap()
    d_ones = nc.dram_tensor("ones_row", [1, BC], BF16,
                            kind="ExternalInput").ap()
    d_eye2 = nc.dram_tensor("eye2", [BC, BC], BF16,
                            kind="ExternalInput").ap()
    d_fcw = nc.dram_tensor("fcw", [BC, 512], F32, kind="ExternalInput").ap()
    d_out = nc.dram_tensor("out", [BC, 1], F32, kind="ExternalOutput").ap()

    with tile.TileContext(nc) as tc, ExitStack() as ctx:
        _body(tc, ctx, nsteps, d_x, d_w, d_wb, d_ones, d_eye2, d_fcw, d_out)
    nc.compile()
    _CACHE[nsteps] = nc
    return nc


def _body(tc, ctx, nsteps, d_x, d_w, d_wb, d_ones, d_eye2, d_fcw, d_out):
    nc = tc.nc
    const = ctx.enter_context(tc.tile_pool(name="const", bufs=1))
    xpool = ctx.enter_context(tc.tile_pool(name="x", bufs=4))
    gact = ctx.enter_context(tc.tile_pool(name="gact", bufs=2))
    state = ctx.enter_context(tc.tile_pool(name="state", bufs=2))
    # Per-strip PSUM tiles: Tile's dependency tracking is per-tile, so
    # separate tiles let each activation start as soon as its own
    # strip's accumulation finishes. {f,i} share one bank-wide tile
    # (both sigmoid -> a single [128,512] ACT op). Single-buffered ON
    # PURPOSE: the WAR dep (next step's bias waits on this step's
    # activation read) paces the PE across the elementwise window,
    # which keeps HAM from re-throttling the clock (bufs=2 measured
    # 555us of cold-clock vs 69us).
    psFI = ctx.enter_context(tc.tile_pool(name="psFI", bufs=1, space="PSUM"))
    psC = ctx.enter_context(tc.tile_pool(name="psC", bufs=1, space="PSUM"))
    psO = ctx.enter_context(tc.tile_pool(name="psO", bufs=1, space="PSUM"))
    psTa = ctx.enter_context(tc.tile_pool(name="psTa", bufs=1, space="PSUM"))
    psTb = ctx.enter_context(tc.tile_pool(name="psTb", bufs=1, space="PSUM"))

    sW = const.tile([128, NK, 4 * H], BF16)
    nc.sync.dma_start(out=sW[:], in_=d_w.rearrange("k p n -> p k n"))
    sWb = const.tile([1, 2, 1024], BF16)
    nc.sync.dma_start(out=sWb[:], in_=d_wb)
    s_ones = const.tile([1, BC], BF16)
    nc.sync.dma_start(out=s_ones[:], in_=d_ones)
    s_eye2 = const.tile([BC, BC], BF16)
    nc.sync.dma_start(out=s_eye2[:], in_=d_eye2)
    s_fcw = const.tile([BC, 512], F32)
    nc.sync.dma_start(out=s_fcw[:], in_=d_fcw)

    c_prev = state.tile([128, 256], F32, tag="c")
    nc.vector.memset(c_prev[:], 0.0)

    def gate_out(strips, g, ph):
        # gate g -> (tile, col offset): f,i share the FI tile
        fi, c, o = strips
        ps, co = (fi, 256 * g) if g < 2 else (c, 0) if g == 2 else (o, 0)
        return ps[64 * ph:64 * ph + 64, co:co + 256]

    def emit_bias_x(t, strips, stop_after_x):
        xs = xpool.tile([128, KX, BC], BF16, tag="xs")
        nc.sync.dma_start(out=xs[:], in_=d_x[t])
        fi, c, o = strips
        # bias first (paced by the previous step's activation reads via
        # WAR on the single-buffered banks), then x g-major.
        for ph in range(2):
            nc.tensor.matmul(fi[64 * ph:64 * ph + 64, :], s_ones[:],
                             sWb[:, ph, 0:512], start=True, stop=False)
        for ph in range(2):
            nc.tensor.matmul(c[64 * ph:64 * ph + 64, :], s_ones[:],
                             sWb[:, ph, 512:768], start=True, stop=False)
        for ph in range(2):
            nc.tensor.matmul(o[64 * ph:64 * ph + 64, :], s_ones[:],
                             sWb[:, ph, 768:1024], start=True, stop=False)
        for g in SORD:
            for kx in range(KX):
                for ph in range(2):
                    rhs = sW[:, KH + kx, 512 * g + 256 * ph:
                             512 * g + 256 * ph + 256]
                    nc.tensor.matmul(gate_out(strips, g, ph),
                                     xs[:, kx, :], rhs, start=False,
                                     stop=(stop_after_x and kx == KX - 1))

    def emit_one_h(strips, g, k, hTa, hTb):
        hT = hTa if k < 2 else hTb
        for ph in range(2):
            rhs = sW[:, k, 512 * g + 256 * ph:512 * g + 256 * ph + 256]
            nc.tensor.matmul(gate_out(strips, g, ph), hT[:, k % 2, :],
                             rhs, start=False, stop=(k == 3))

    def emit_h_all(strips, hTa, hTb):
        # fi tile first (completes after 8 pairs -> its sigmoid starts
        # earliest); within {f,i} do the hTa-dependent ks for both
        # gates before the hTb ones so the first pairs never stall on
        # copy_b. c and o strips follow strip-major.
        for g in (0, 1):
            for k in (0, 1):
                emit_one_h(strips, g, k, hTa, hTb)
        for g in (0, 1):
            for k in (2, 3):
                emit_one_h(strips, g, k, hTa, hTb)
        for g in (2, 3):
            for k in range(KH):
                emit_one_h(strips, g, k, hTa, hTb)

    def new_strips():
        fi = psFI.tile([128, 512], F32, tag="FI")
        c = psC.tile([128, 256], F32, tag="C")
        o = psO.tile([128, 256], F32, tag="O")
        return (fi, c, o)

    # prologue: gates(0) = bias + x only
    strips = new_strips()
    emit_bias_x(0, strips, stop_after_x=True)

    h_lo = h_hi = None
    for t in range(nsteps):
        last = t == nsteps - 1
        if not last:
            strips_n = new_strips()
            emit_bias_x(t + 1, strips_n, stop_after_x=False)

        # elementwise for step t; strips = (fi, c, o)
        fi_t, c_t, o_t = strips
        sfi = gact.tile([128, 512], F32, tag="sfi")
        nc.scalar.activation(sfi[:], fi_t[:], AF.Sigmoid)
        tc_ = gact.tile([128, 256], F32, tag="tc")
        nc.scalar.activation(tc_[:], c_t[:], AF.Tanh)
        so = gact.tile([128, 256], BF16, tag="so")
        nc.scalar.activation(so[:], o_t[:], AF.Sigmoid)

        u1 = gact.tile([128, 256], F32, tag="u1")
        nc.vector.tensor_mul(u1[:], c_prev[:], sfi[:, 0:256])
        u2 = gact.tile([128, 256], F32, tag="u2")
        nc.vector.tensor_mul(u2[:], sfi[:, 256:512], tc_[:])
        c_new = state.tile([128, 256], F32, tag="c")
        nc.vector.tensor_add(c_new[:], u1[:], u2[:])
        tch = gact.tile([128, 256], BF16, tag="tch")
        nc.scalar.activation(tch[:], c_new[:], AF.Tanh)
        # h split by hidden half, both landed at partition base 0
        # (engines support partition-shifted reads) so the PE transposes
        # only see base-0 stationaries (base-64 ones crash the PE).
        h_lo = state.tile([BC, 256], BF16, tag="hlo")    # hid 0:256
        nc.vector.tensor_mul(h_lo[:], so[0:64, :], tch[0:64, :])
        c_prev = c_new

        if not last:
            # transpose+copy chunks (0,1) from h_lo BEFORE the h_hi mul
            # so copy_a isn't queued behind it on the DVE and wave (0,1)
            # starts earlier.
            pTa = psTa.tile([128, 2, BC], BF16, tag="pTa")
            nc.tensor.transpose(pTa[:, 0, :], h_lo[:, 0:128], s_eye2[:])
            nc.tensor.transpose(pTa[:, 1, :], h_lo[:, 128:256], s_eye2[:])
            hTa = state.tile([128, 2, BC], BF16, tag="hTa")
            nc.vector.tensor_copy(hTa[:], pTa[:])
            h_hi = state.tile([BC, 256], BF16, tag="hhi")    # hid 256:512
            nc.vector.tensor_mul(h_hi[:], so[64:128, :], tch[64:128, :])
            pTb = psTb.tile([128, 2, BC], BF16, tag="pTb")
            nc.tensor.transpose(pTb[:, 0, :], h_hi[:, 0:128], s_eye2[:])
            nc.tensor.transpose(pTb[:, 1, :], h_hi[:, 128:256], s_eye2[:])
            hTb = state.tile([128, 2, BC], BF16, tag="hTb")
            nc.vector.tensor_copy(hTb[:], pTb[:])
            emit_h_all(strips_n, hTa, hTb)
            strips = strips_n
        else:
            h_hi = state.tile([BC, 256], BF16, tag="hhi")
            nc.vector.tensor_mul(h_hi[:], so[64:128, :], tch[64:128, :])

    # fc head: out[b] = sum_u h[b, u] * fc_w[u]; host adds fc_b
    ra = gact.tile([BC, 1], F32, tag="ra")
    ma = gact.tile([BC, 256], F32, tag="ma")
    nc.vector.scalar_tensor_tensor(ma[:], h_lo[:], 1.0, s_fcw[:, 0:256],
                                   op0=ALU.mult, op1=ALU.mult,
                                   accum_out=ra[:])
    rb = gact.tile([BC, 1], F32, tag="rb")
    mb = gact.tile([BC, 256], F32, tag="mb")
    nc.vector.scalar_tensor_tensor(mb[:], h_hi[:], 1.0, s_fcw[:, 256:512],
                                   op0=ALU.mult, op1=ALU.mult,
                                   accum_out=rb[:])
    ro = gact.tile([BC, 1], F32, tag="ro")
    nc.vector.tensor_add(ro[:], ra[:], rb[:])
    nc.sync.dma_start(out=d_out, in_=ro[:])


def _prep_core_inputs(x, W_w, W_b, fc_w, fc_b, core, nsteps=T):
    """Host-side shard + relayout for one core."""
    xs = x[core * BC:(core + 1) * BC, :nsteps]          # [BC, t, I]
    xt = np.ascontiguousarray(xs.transpose(1, 2, 0))    # [t, I, BC]
    xt = xt.reshape(nsteps, KX, 128, BC).transpose(0, 2, 1, 3)
    xt = np.ascontiguousarray(xt)                       # [t, 128, KX, BC]

    # gate reorder (f, i, o, c_hat) -> (f, i, c_hat, o)
    perm = np.concatenate([np.arange(0, 1024), np.arange(1536, 2048),
                           np.arange(1024, 1536)])
    WT = W_w.T[:, perm]                                 # [768, 2048]
    wt = np.ascontiguousarray(WT.reshape(NK, 128, 4 * H))
    # bias cols per ph: [f 256 | i 256 | c 256 | o 256] -> [1, 2, 1024]
    wb_re = np.ascontiguousarray(
        W_b[perm].reshape(4, 2, 256).transpose(1, 0, 2).reshape(1, 2, 1024))

    ones_row = np.ones((1, BC), dtype=np.float32)
    eye2 = np.eye(BC, dtype=np.float32)
    fcw = np.ascontiguousarray(
        np.broadcast_to(fc_w.reshape(1, H), (BC, H)).astype(np.float32))

    import ml_dtypes
    bf = ml_dtypes.bfloat16
    return {"xT": xt.astype(bf), "W": wt.astype(bf),
            "Wb": wb_re.astype(bf), "ones_row": ones_row.astype(bf),
            "eye2": eye2.astype(bf), "fcw": fcw}


def kernel(x, W_w, W_b, fc_w, fc_b):
    x = np.asarray(x, dtype=np.float32)
    W_w = np.asarray(W_w, dtype=np.float32)
    W_b = np.asarray(W_b, dtype=np.float32)
    fc_w = np.asarray(fc_w, dtype=np.float32)
    fc_b = np.asarray(fc_b, dtype=np.float32)

    nc = _build(T)
    in_maps = [_prep_core_inputs(x, W_w, W_b, fc_w, fc_b, c)
               for c in range(NCORES)]
    res = run_bass_kernel_spmd(nc, in_maps, list(range(NCORES))).results
    return np.concatenate(
        [res[c]["out"] + np.float32(fc_b[0]) for c in range(NCORES)], axis=0)


# revision 55
# speedup vs baseline: 1.1045x; 1.0012x over previous
"""Trainium2 Bass kernel for CustomLSTMForecast.

B=512, T=256, I=256, H=512. Data-parallel: batch sharded 8 ways (64
rows/core), LSTM + fc weights replicated. 1.23 ms HW (baseline 1.92).

Per-core design (batch m = 64):

Cell-state layout ("c-layout") [128, 256]: partition p = b + 64*ph
(b = batch row, ph = hidden half), column j = hidden unit u - 256*ph,
so every elementwise op uses all 128 partitions.

Gates accumulate in three per-step PSUM tiles (each its own bank, so
Tile's per-tile dep tracking releases each activation as soon as its
own strips finish): FI [128,512] (f cols 0:256, i 256:512 -> ONE
[128,512] sigmoid), C [128,256] (tanh), O [128,256] (sigmoid). Each
256-col strip: bias (K=1 ones row, start=True) + 2 x-chunks + 4
h-chunks (stop on k=3), all N=256 matmuls; the two partition halves
(ph) pair up in PE column groups (auto tile_position from
out.base_partition in {0,64}) so two M=64 matmuls run concurrently.

Gate pools are single-buffered ON PURPOSE: the WAR dep (bias of step
t+1 waits on step t's activation read) paces the PE's bias/x stream
across the elementwise window, which keeps the PE from idling long
enough for HAM to re-throttle the clock (bufs=2 measured 555us of
cold-clock vs 69us, and was net slower).

h is produced as two bf16 [64,256] tiles at partition base 0 via
partition-shifted DVE muls (h_lo = hid 0:256, h_hi = 256:512) because
PE transposes reject base-64 stationaries. PE-transposed into
hTa = chunks (0,1), hTb = (2,3); h-matmuls are emitted fi-tile-first,
hTa-ks before hTb-ks, so the fi sigmoid starts after 8 wave pairs and
nothing stalls on the second copy.
"""
from contextlib import ExitStack

import numpy as np

import concourse.bass as bass
import concourse.tile as tile
from concourse import bacc, mybir
from concourse.bass_utils import run_bass_kernel_spmd

F32 = mybir.dt.float32
BF16 = mybir.dt.bfloat16
AF = mybir.ActivationFunctionType
ALU = mybir.AluOpType

B, T, I, H = 512, 256, 256, 512
NCORES = 8
BC = B // NCORES          # 64 batch rows per core
KH = H // 128             # 4 hidden k-chunks
KX = I // 128             # 2 input k-chunks
NK = KH + KX              # 6 contraction chunks (no bias chunk)

# gate order in the repacked W: g=0 f, 1 i, 2 c_hat, 3 o.
# W_w row blocks are (f, i, o, c_hat) -> col-reorder [0:1024, 1536:2048,
# 1024:1536].
SORD = (0, 1, 2, 3)       # strip completion / emission order: f, i, c, o

_CACHE = {}


def _build(nsteps=T):
    if nsteps in _CACHE:
        return _CACHE[nsteps]
    nc = bacc.Bacc("TRN2", target_bir_lowering=False, debug=False,
                   num_devices=NCORES)
    d_x = nc.dram_tensor("xT", [nsteps, 128, KX, BC], BF16,
                         kind="ExternalInput").ap()
    d_w = nc.dram_tensor("W", [NK, 128, 4 * H], BF16,
                         kind="ExternalInput").ap()
    d_wb = nc.dram_tensor("Wb", [1, 2, 1024], BF16,
                          kind="ExternalInput").ap()
    d_ones = nc.dram_tensor("ones_row", [1, BC], BF16,
                            kind="ExternalInput").ap()
    d_eye2 = nc.dram_tensor("eye2", [BC, BC], BF16,
                            kind="ExternalInput").ap()
    d_fcw = nc.dram_tensor("fcw", [BC, 512], F32, kind="ExternalInput").ap()
    d_out = nc.dram_tensor("out", [BC, 1], F32, kind="ExternalOutput").ap()

    with tile.TileContext(nc) as tc, ExitStack() as ctx:
        _body(tc, ctx, nsteps, d_x, d_w, d_wb, d_ones, d_eye2, d_fcw, d_out)
    nc.compile()
    _CACHE[nsteps] = nc
    return nc


def _body(tc, ctx, nsteps, d_x, d_w, d_wb, d_ones, d_eye2, d_fcw, d_out):
    nc = tc.nc
    const = ctx.enter_context(tc.tile_pool(name="const", bufs=1))
    xpool = ctx.enter_context(tc.tile_pool(name="x", bufs=4))
    gact = ctx.enter_context(tc.tile_pool(name="gact", bufs=2))
    state = ctx.enter_context(tc.tile_pool(name="state", bufs=2))
    # Per-strip PSUM tiles: Tile's dependency tracking is per-tile, so
    # separate tiles let each activation start as soon as its own
    # strip's accumulation finishes. {f,i} share one bank-wide tile
    # (both sigmoid -> a single [128,512] ACT op). Single-buffered ON
    # PURPOSE: the WAR dep (next step's bias waits on this step's
    # activation read) paces the PE across the elementwise window,
    # which keeps HAM from re-throttling the clock (bufs=2 measured
    # 555us of cold-clock vs 69us).
    psFI = ctx.enter_context(tc.tile_pool(name="psFI", bufs=1, space="PSUM"))
    psC = ctx.enter_context(tc.tile_pool(name="psC", bufs=1, space="PSUM"))
    psO = ctx.enter_context(tc.tile_pool(name="psO", bufs=1, space="PSUM"))
    psT0 = ctx.enter_context(tc.tile_pool(name="psT0", bufs=1, space="PSUM"))
    psT1 = ctx.enter_context(tc.tile_pool(name="psT1", bufs=1, space="PSUM"))
    psTb = ctx.enter_context(tc.tile_pool(name="psTb", bufs=1, space="PSUM"))

    sW = const.tile([128, NK, 4 * H], BF16)
    nc.sync.dma_start(out=sW[:], in_=d_w.rearrange("k p n -> p k n"))
    sWb = const.tile([1, 2, 1024], BF16)
    nc.sync.dma_start(out=sWb[:], in_=d_wb)
    s_ones = const.tile([1, BC], BF16)
    nc.sync.dma_start(out=s_ones[:], in_=d_ones)
    s_eye2 = const.tile([BC, BC], BF16)
    nc.sync.dma_start(out=s_eye2[:], in_=d_eye2)
    s_fcw = const.tile([BC, 512], F32)
    nc.sync.dma_start(out=s_fcw[:], in_=d_fcw)

    c_a = state.tile([128, 128], F32, tag="ca")
    nc.vector.memset(c_a[:], 0.0)
    c_b = state.tile([128, 128], F32, tag="cb")
    nc.vector.memset(c_b[:], 0.0)
    c_prev = (c_a, c_b)

    def gate_out(strips, g, ph):
        # gate g -> (tile, col offset): f,i share the FI tile
        fi, c, o = strips
        ps, co = (fi, 256 * g) if g < 2 else (c, 0) if g == 2 else (o, 0)
        return ps[64 * ph:64 * ph + 64, co:co + 256]

    def emit_bias_x(t, strips, stop_after_x):
        xs = xpool.tile([128, KX, BC], BF16, tag="xs")
        nc.sync.dma_start(out=xs[:], in_=d_x[t])
        fi, c, o = strips
        # bias first (paced by the previous step's activation reads via
        # WAR on the single-buffered banks), then x g-major.
        for ph in range(2):
            nc.tensor.matmul(fi[64 * ph:64 * ph + 64, :], s_ones[:],
                             sWb[:, ph, 0:512], start=True, stop=False)
        for ph in range(2):
            nc.tensor.matmul(c[64 * ph:64 * ph + 64, :], s_ones[:],
                             sWb[:, ph, 512:768], start=True, stop=False)
        for ph in range(2):
            nc.tensor.matmul(o[64 * ph:64 * ph + 64, :], s_ones[:],
                             sWb[:, ph, 768:1024], start=True, stop=False)
        for g in SORD:
            for kx in range(KX):
                for ph in range(2):
                    rhs = sW[:, KH + kx, 512 * g + 256 * ph:
                             512 * g + 256 * ph + 256]
                    nc.tensor.matmul(gate_out(strips, g, ph),
                                     xs[:, kx, :], rhs, start=False,
                                     stop=(stop_after_x and kx == KX - 1))

    def emit_one_h(strips, g, k, hTs):
        for ph in range(2):
            rhs = sW[:, k, 512 * g + 256 * ph:512 * g + 256 * ph + 256]
            nc.tensor.matmul(gate_out(strips, g, ph), hTs[k],
                             rhs, start=False, stop=(k == 3))

    def emit_h_all(strips, hTs):
        # fi tile first (completes after 8 pairs -> its sigmoid starts
        # earliest), k-major within it so the first pairs only need the
        # earliest hT chunk copies. c and o strips follow strip-major.
        for k in range(KH):
            for g in (0, 1):
                emit_one_h(strips, g, k, hTs)
        for g in (2, 3):
            for k in range(KH):
                emit_one_h(strips, g, k, hTs)

    def new_strips():
        fi = psFI.tile([128, 512], F32, tag="FI")
        c = psC.tile([128, 256], F32, tag="C")
        o = psO.tile([128, 256], F32, tag="O")
        return (fi, c, o)

    # prologue: gates(0) = bias + x only
    strips = new_strips()
    emit_bias_x(0, strips, stop_after_x=True)

    h_lo = h_hi = None
    for t in range(nsteps):
        last = t == nsteps - 1
        if not last:
            strips_n = new_strips()
            emit_bias_x(t + 1, strips_n, stop_after_x=False)

        # elementwise for step t; strips = (fi, c, o)
        fi_t, c_t, o_t = strips
        sfi = gact.tile([128, 512], F32, tag="sfi")
        nc.scalar.activation(sfi[:], fi_t[:], AF.Sigmoid)
        tc_ = gact.tile([128, 256], F32, tag="tc")
        nc.scalar.activation(tc_[:], c_t[:], AF.Tanh)
        so = gact.tile([128, 256], BF16, tag="so")
        nc.scalar.activation(so[:], o_t[:], AF.Sigmoid)

        # c-update split into column halves so the tanh/h/transpose tail
        # wavefronts: half 0 covers hid chunks 0 (p<64) and 2 (p>=64).
        ca_p, cb_p = c_prev
        u1a = gact.tile([128, 128], F32, tag="u1a")
        nc.vector.tensor_mul(u1a[:], ca_p[:], sfi[:, 0:128])
        u1b = gact.tile([128, 128], F32, tag="u1b")
        nc.vector.tensor_mul(u1b[:], cb_p[:], sfi[:, 128:256])
        u2a = gact.tile([128, 128], F32, tag="u2a")
        nc.vector.tensor_mul(u2a[:], sfi[:, 256:384], tc_[:, 0:128])
        c_a = state.tile([128, 128], F32, tag="ca")
        nc.vector.tensor_add(c_a[:], u1a[:], u2a[:])
        u2b = gact.tile([128, 128], F32, tag="u2b")
        nc.vector.tensor_mul(u2b[:], sfi[:, 384:512], tc_[:, 128:256])
        c_b = state.tile([128, 128], F32, tag="cb")
        nc.vector.tensor_add(c_b[:], u1b[:], u2b[:])
        tch_a = gact.tile([128, 128], BF16, tag="tcha")
        nc.scalar.activation(tch_a[:], c_a[:], AF.Tanh)
        tch_b = gact.tile([128, 128], BF16, tag="tchb")
        nc.scalar.activation(tch_b[:], c_b[:], AF.Tanh)
        c_prev = (c_a, c_b)

        # h quarters at partition base 0 (engines support partition-
        # shifted reads; the PE transposes need base-0 stationaries).
        # DVE does the h_lo halves (feed the first wave pairs), GpSimd
        # the h_hi halves in parallel.
        hl0 = state.tile([BC, 128], BF16, tag="hl0")     # hid 0:128
        nc.vector.tensor_mul(hl0[:], so[0:64, 0:128], tch_a[0:64, :])
        hl1 = state.tile([BC, 128], BF16, tag="hl1")     # hid 128:256
        nc.vector.tensor_mul(hl1[:], so[0:64, 128:256], tch_b[0:64, :])
        hh0 = state.tile([BC, 128], BF16, tag="hh0")     # hid 256:384
        nc.gpsimd.tensor_mul(hh0[:], so[64:128, 0:128], tch_a[64:128, :])
        hh1 = state.tile([BC, 128], BF16, tag="hh1")     # hid 384:512
        nc.gpsimd.tensor_mul(hh1[:], so[64:128, 128:256], tch_b[64:128, :])

        if not last:
            pT0 = psT0.tile([128, BC], BF16, tag="pT0")
            nc.tensor.transpose(pT0[:], hl0[:], s_eye2[:])
            hT0 = state.tile([128, BC], BF16, tag="hT0")
            nc.vector.tensor_copy(hT0[:], pT0[:])
            pT1 = psT1.tile([128, BC], BF16, tag="pT1")
            nc.tensor.transpose(pT1[:], hl1[:], s_eye2[:])
            hT1 = state.tile([128, BC], BF16, tag="hT1")
            nc.vector.tensor_copy(hT1[:], pT1[:])
            pTb = psTb.tile([128, 2, BC], BF16, tag="pTb")
            nc.tensor.transpose(pTb[:, 0, :], hh0[:], s_eye2[:])
            nc.tensor.transpose(pTb[:, 1, :], hh1[:], s_eye2[:])
            hTb = state.tile([128, 2, BC], BF16, tag="hTb")
            nc.vector.tensor_copy(hTb[:], pTb[:])
            emit_h_all(strips_n,
                       (hT0[:], hT1[:], hTb[:, 0, :], hTb[:, 1, :]))
            strips = strips_n

    # fc head: out[b] = sum_u h[b, u] * fc_w[u]; host adds fc_b
    hq = (hl0, hl1, hh0, hh1)
    rs = []
    for q in range(4):
        r = gact.tile([BC, 1], F32, tag=f"r{q}")
        m = gact.tile([BC, 128], F32, tag=f"m{q}")
        nc.vector.scalar_tensor_tensor(m[:], hq[q][:], 1.0,
                                       s_fcw[:, 128 * q:128 * q + 128],
                                       op0=ALU.mult, op1=ALU.mult,
                                       accum_out=r[:])
        rs.append(r)
    rab = gact.tile([BC, 1], F32, tag="rab")
    nc.vector.tensor_add(rab[:], rs[0][:], rs[1][:])
    rcd = gact.tile([BC, 1], F32, tag="rcd")
    nc.vector.tensor_add(rcd[:], rs[2][:], rs[3][:])
    ro = gact.tile([BC, 1], F32, tag="ro")
    nc.vector.tensor_add(ro[:], rab[:], rcd[:])
    nc.sync.dma_start(out=d_out, in_=ro[:])


def _prep_core_inputs(x, W_w, W_b, fc_w, fc_b, core, nsteps=T):
    """Host-side shard + relayout for one core."""
    xs = x[core * BC:(core + 1) * BC, :nsteps]          # [BC, t, I]
    xt = np.ascontiguousarray(xs.transpose(1, 2, 0))    # [t, I, BC]
    xt = xt.reshape(nsteps, KX, 128, BC).transpose(0, 2, 1, 3)
    xt = np.ascontiguousarray(xt)                       # [t, 128, KX, BC]

    # gate reorder (f, i, o, c_hat) -> (f, i, c_hat, o)
    perm = np.concatenate([np.arange(0, 1024), np.arange(1536, 2048),
                           np.arange(1024, 1536)])
    WT = W_w.T[:, perm]                                 # [768, 2048]
    wt = np.ascontiguousarray(WT.reshape(NK, 128, 4 * H))
    # bias cols per ph: [f 256 | i 256 | c 256 | o 256] -> [1, 2, 1024]
    wb_re = np.ascontiguousarray(
        W_b[perm].reshape(4, 2, 256).transpose(1, 0, 2).reshape(1, 2, 1024))

    ones_row = np.ones((1, BC), dtype=np.float32)
    eye2 = np.eye(BC, dtype=np.float32)
    fcw = np.ascontiguousarray(
        np.broadcast_to(fc_w.reshape(1, H), (BC, H)).astype(np.float32))

    import ml_dtypes
    bf = ml_dtypes.bfloat16
    return {"xT": xt.astype(bf), "W": wt.astype(bf),
            "Wb": wb_re.astype(bf), "ones_row": ones_row.astype(bf),
            "eye2": eye2.astype(bf), "fcw": fcw}


def kernel(x, W_w, W_b, fc_w, fc_b):
    x = np.asarray(x, dtype=np.float32)
    W_w = np.asarray(W_w, dtype=np.float32)
    W_b = np.asarray(W_b, dtype=np.float32)
    fc_w = np.asarray(fc_w, dtype=np.float32)
    fc_b = np.asarray(fc_b, dtype=np.float32)

    nc = _build(T)
    in_maps = [_prep_core_inputs(x, W_w, W_b, fc_w, fc_b, c)
               for c in range(NCORES)]
    res = run_bass_kernel_spmd(nc, in_maps, list(range(NCORES))).results
    return np.concatenate(
        [res[c]["out"] + np.float32(fc_b[0]) for c in range(NCORES)], axis=0)


# revision 56
# speedup vs baseline: 1.1219x; 1.0158x over previous
"""Trainium2 Bass kernel for CustomLSTMForecast.

B=512, T=256, I=256, H=512. Data-parallel: batch sharded 8 ways (64
rows/core), LSTM + fc weights replicated. 1.23 ms HW (baseline 1.92).

Per-core design (batch m = 64):

Cell-state layout ("c-layout") [128, 256]: partition p = b + 64*ph
(b = batch row, ph = hidden half), column j = hidden unit u - 256*ph,
so every elementwise op uses all 128 partitions.

Gates accumulate in three per-step PSUM tiles (each its own bank, so
Tile's per-tile dep tracking releases each activation as soon as its
own strips finish): FI [128,512] (f cols 0:256, i 256:512 -> ONE
[128,512] sigmoid), C [128,256] (tanh), O [128,256] (sigmoid). Each
256-col strip: bias (K=1 ones row, start=True) + 2 x-chunks + 4
h-chunks (stop on k=3), all N=256 matmuls; the two partition halves
(ph) pair up in PE column groups (auto tile_position from
out.base_partition in {0,64}) so two M=64 matmuls run concurrently.

Gate pools are single-buffered ON PURPOSE: the WAR dep (bias of step
t+1 waits on step t's activation read) paces the PE's bias/x stream
across the elementwise window, which keeps the PE from idling long
enough for HAM to re-throttle the clock (bufs=2 measured 555us of
cold-clock vs 69us, and was net slower).

h is produced as two bf16 [64,256] tiles at partition base 0 via
partition-shifted DVE muls (h_lo = hid 0:256, h_hi = 256:512) because
PE transposes reject base-64 stationaries. PE-transposed into
hTa = chunks (0,1), hTb = (2,3); h-matmuls are emitted fi-tile-first,
hTa-ks before hTb-ks, so the fi sigmoid starts after 8 wave pairs and
nothing stalls on the second copy.
"""
from contextlib import ExitStack

import numpy as np

import concourse.bass as bass
import concourse.tile as tile
from concourse import bacc, mybir
from concourse.bass_utils import run_bass_kernel_spmd

F32 = mybir.dt.float32
BF16 = mybir.dt.bfloat16
AF = mybir.ActivationFunctionType
ALU = mybir.AluOpType

B, T, I, H = 512, 256, 256, 512
NCORES = 8
BC = B // NCORES          # 64 batch rows per core
KH = H // 128             # 4 hidden k-chunks
KX = I // 128             # 2 input k-chunks
NK = KH + KX              # 6 contraction chunks (no bias chunk)

# gate order in the repacked W: g=0 f, 1 i, 2 c_hat, 3 o.
# W_w row blocks are (f, i, o, c_hat) -> col-reorder [0:1024, 1536:2048,
# 1024:1536].
SORD = (0, 1, 2, 3)       # strip completion / emission order: f, i, c, o

_CACHE = {}


def _build(nsteps=T):
    if nsteps in _CACHE:
        return _CACHE[nsteps]
    nc = bacc.Bacc("TRN2", target_bir_lowering=False, debug=False,
                   num_devices=NCORES)
    d_x = nc.dram_tensor("xT", [nsteps, 128, KX, BC], BF16,
                         kind="ExternalInput").ap()
    d_w = nc.dram_tensor("W", [NK, 128, 4 * H], BF16,
                         kind="ExternalInput").ap()
    d_wb = nc.dram_tensor("Wb", [1, 2, 1024], BF16,
                          kind="ExternalInput").ap()
    d_ones = nc.dram_tensor("ones_row", [1, BC], BF16,
                            kind="ExternalInput").ap()
    d_eye2 = nc.dram_tensor("eye2", [BC, BC], BF16,
                            kind="ExternalInput").ap()
    d_fcw = nc.dram_tensor("fcw", [BC, 512], F32, kind="ExternalInput").ap()
    d_out = nc.dram_tensor("out", [BC, 1], F32, kind="ExternalOutput").ap()

    with tile.TileContext(nc) as tc, ExitStack() as ctx:
        _body(tc, ctx, nsteps, d_x, d_w, d_wb, d_ones, d_eye2, d_fcw, d_out)
    nc.compile()
    _CACHE[nsteps] = nc
    return nc


def _body(tc, ctx, nsteps, d_x, d_w, d_wb, d_ones, d_eye2, d_fcw, d_out):
    nc = tc.nc
    const = ctx.enter_context(tc.tile_pool(name="const", bufs=1))
    xpool = ctx.enter_context(tc.tile_pool(name="x", bufs=4))
    gact = ctx.enter_context(tc.tile_pool(name="gact", bufs=2))
    state = ctx.enter_context(tc.tile_pool(name="state", bufs=2))
    # Per-strip PSUM tiles: Tile's dependency tracking is per-tile, so
    # separate tiles let each activation start as soon as its own
    # strip's accumulation finishes. {f,i} share one bank-wide tile
    # (both sigmoid -> a single [128,512] ACT op). Single-buffered ON
    # PURPOSE: the WAR dep (next step's bias waits on this step's
    # activation read) paces the PE across the elementwise window,
    # which keeps HAM from re-throttling the clock (bufs=2 measured
    # 555us of cold-clock vs 69us).
    psFI = ctx.enter_context(tc.tile_pool(name="psFI", bufs=1, space="PSUM"))
    psC = ctx.enter_context(tc.tile_pool(name="psC", bufs=1, space="PSUM"))
    psO = ctx.enter_context(tc.tile_pool(name="psO", bufs=1, space="PSUM"))
    psT0 = ctx.enter_context(tc.tile_pool(name="psT0", bufs=1, space="PSUM"))
    psT1 = ctx.enter_context(tc.tile_pool(name="psT1", bufs=1, space="PSUM"))
    psTb = ctx.enter_context(tc.tile_pool(name="psTb", bufs=1, space="PSUM"))

    sW = const.tile([128, NK, 4 * H], BF16)
    nc.sync.dma_start(out=sW[:], in_=d_w.rearrange("k p n -> p k n"))
    sWb = const.tile([1, 2, 1024], BF16)
    nc.sync.dma_start(out=sWb[:], in_=d_wb)
    s_ones = const.tile([1, BC], BF16)
    nc.sync.dma_start(out=s_ones[:], in_=d_ones)
    s_eye2 = const.tile([BC, BC], BF16)
    nc.sync.dma_start(out=s_eye2[:], in_=d_eye2)
    s_fcw = const.tile([BC, 512], F32)
    nc.sync.dma_start(out=s_fcw[:], in_=d_fcw)

    c_a = state.tile([128, 128], F32, tag="ca")
    nc.vector.memset(c_a[:], 0.0)
    c_b = state.tile([128, 128], F32, tag="cb")
    nc.vector.memset(c_b[:], 0.0)
    c_prev = (c_a, c_b)

    def gate_out(strips, g, ph):
        # gate g -> (tile, col offset): f,i share the FI tile
        fi, c, o = strips
        ps, co = (fi, 256 * g) if g < 2 else (c, 0) if g == 2 else (o, 0)
        return ps[64 * ph:64 * ph + 64, co:co + 256]

    def emit_bias_x(t, strips, stop_after_x):
        xs = xpool.tile([128, KX, BC], BF16, tag="xs")
        nc.sync.dma_start(out=xs[:], in_=d_x[t])
        fi, c, o = strips
        # bias first (paced by the previous step's activation reads via
        # WAR on the single-buffered banks), then x g-major.
        for ph in range(2):
            nc.tensor.matmul(fi[64 * ph:64 * ph + 64, :], s_ones[:],
                             sWb[:, ph, 0:512], start=True, stop=False)
        for ph in range(2):
            nc.tensor.matmul(c[64 * ph:64 * ph + 64, :], s_ones[:],
                             sWb[:, ph, 512:768], start=True, stop=False)
        for ph in range(2):
            nc.tensor.matmul(o[64 * ph:64 * ph + 64, :], s_ones[:],
                             sWb[:, ph, 768:1024], start=True, stop=False)
        for g in SORD:
            for kx in range(KX):
                for ph in range(2):
                    rhs = sW[:, KH + kx, 512 * g + 256 * ph:
                             512 * g + 256 * ph + 256]
                    nc.tensor.matmul(gate_out(strips, g, ph),
                                     xs[:, kx, :], rhs, start=False,
                                     stop=(stop_after_x and kx == KX - 1))

    def emit_one_h(strips, g, k, hTs):
        for ph in range(2):
            rhs = sW[:, k, 512 * g + 256 * ph:512 * g + 256 * ph + 256]
            nc.tensor.matmul(gate_out(strips, g, ph), hTs[k],
                             rhs, start=False, stop=(k == 3))

    def emit_h_all(strips, hTs):
        # fi tile first (completes after 8 pairs -> its sigmoid starts
        # earliest), k-major within it so the first pairs only need the
        # earliest hT chunk copies. c and o strips follow strip-major.
        for k in range(KH):
            for g in (0, 1):
                emit_one_h(strips, g, k, hTs)
        for g in (2, 3):
            for k in range(KH):
                emit_one_h(strips, g, k, hTs)

    def new_strips():
        fi = psFI.tile([128, 512], F32, tag="FI")
        c = psC.tile([128, 256], F32, tag="C")
        o = psO.tile([128, 256], F32, tag="O")
        return (fi, c, o)

    # prologue: gates(0) = bias + x only
    strips = new_strips()
    emit_bias_x(0, strips, stop_after_x=True)

    h_lo = h_hi = None
    for t in range(nsteps):
        last = t == nsteps - 1
        if not last:
            strips_n = new_strips()
            emit_bias_x(t + 1, strips_n, stop_after_x=False)

        # elementwise for step t; strips = (fi, c, o)
        fi_t, c_t, o_t = strips
        sfi = gact.tile([128, 512], F32, tag="sfi")
        nc.scalar.activation(sfi[:], fi_t[:], AF.Sigmoid)
        tc_ = gact.tile([128, 256], F32, tag="tc")
        nc.scalar.activation(tc_[:], c_t[:], AF.Tanh)
        so = gact.tile([128, 256], BF16, tag="so")
        nc.scalar.activation(so[:], o_t[:], AF.Sigmoid)

        # c-update split into column halves so the tanh/h/transpose tail
        # wavefronts: half 0 covers hid chunks 0 (p<64) and 2 (p>=64).
        ca_p, cb_p = c_prev
        u1a = gact.tile([128, 128], F32, tag="u1a")
        nc.vector.tensor_mul(u1a[:], ca_p[:], sfi[:, 0:128])
        u1b = gact.tile([128, 128], F32, tag="u1b")
        nc.vector.tensor_mul(u1b[:], cb_p[:], sfi[:, 128:256])
        u2a = gact.tile([128, 128], F32, tag="u2a")
        nc.vector.tensor_mul(u2a[:], sfi[:, 256:384], tc_[:, 0:128])
        c_a = state.tile([128, 128], F32, tag="ca")
        nc.vector.tensor_add(c_a[:], u1a[:], u2a[:])
        u2b = gact.tile([128, 128], F32, tag="u2b")
        nc.vector.tensor_mul(u2b[:], sfi[:, 384:512], tc_[:, 128:256])
        c_b = state.tile([128, 128], F32, tag="cb")
        nc.vector.tensor_add(c_b[:], u1b[:], u2b[:])
        tch_a = gact.tile([128, 128], BF16, tag="tcha")
        nc.scalar.activation(tch_a[:], c_a[:], AF.Tanh)
        tch_b = gact.tile([128, 128], BF16, tag="tchb")
        nc.scalar.activation(tch_b[:], c_b[:], AF.Tanh)
        c_prev = (c_a, c_b)

        # h quarters at partition base 0 (engines support partition-
        # shifted reads; the PE transposes need base-0 stationaries).
        # DVE does the h_lo halves (feed the first wave pairs), GpSimd
        # the h_hi halves in parallel.
        hl0 = state.tile([BC, 128], BF16, tag="hl0")     # hid 0:128
        nc.vector.tensor_mul(hl0[:], so[0:64, 0:128], tch_a[0:64, :])
        hl1 = state.tile([BC, 128], BF16, tag="hl1")     # hid 128:256
        nc.vector.tensor_mul(hl1[:], so[0:64, 128:256], tch_b[0:64, :])
        hh0 = state.tile([BC, 128], BF16, tag="hh0")     # hid 256:384
        nc.gpsimd.tensor_mul(hh0[:], so[64:128, 0:128], tch_a[64:128, :])
        # hh1 on DVE: it's the last producer before the final transpose
        # (which gates the whole wave stream), and the GpSimd queue is
        # still busy with hh0 (~420ns/op) while the DVE is free (~215ns)
        hh1 = state.tile([BC, 128], BF16, tag="hh1")     # hid 384:512
        nc.vector.tensor_mul(hh1[:], so[64:128, 128:256], tch_b[64:128, :])

        if not last:
            pT0 = psT0.tile([128, BC], BF16, tag="pT0")
            nc.tensor.transpose(pT0[:], hl0[:], s_eye2[:])
            hT0 = state.tile([128, BC], BF16, tag="hT0")
            nc.vector.tensor_copy(hT0[:], pT0[:])
            pT1 = psT1.tile([128, BC], BF16, tag="pT1")
            nc.tensor.transpose(pT1[:], hl1[:], s_eye2[:])
            hT1 = state.tile([128, BC], BF16, tag="hT1")
            nc.vector.tensor_copy(hT1[:], pT1[:])
            pTb = psTb.tile([128, 2, BC], BF16, tag="pTb")
            nc.tensor.transpose(pTb[:, 0, :], hh0[:], s_eye2[:])
            nc.tensor.transpose(pTb[:, 1, :], hh1[:], s_eye2[:])
            hTb = state.tile([128, 2, BC], BF16, tag="hTb")
            nc.vector.tensor_copy(hTb[:], pTb[:])
            emit_h_all(strips_n,
                       (hT0[:], hT1[:], hTb[:, 0, :], hTb[:, 1, :]))
            strips = strips_n

    # fc head: out[b] = sum_u h[b, u] * fc_w[u]; host adds fc_b
    hq = (hl0, hl1, hh0, hh1)
    rs = []
    for q in range(4):
        r = gact.tile([BC, 1], F32, tag=f"r{q}")
        m = gact.tile([BC, 128], F32, tag=f"m{q}")
        nc.vector.scalar_tensor_tensor(m[:], hq[q][:], 1.0,
                                       s_fcw[:, 128 * q:128 * q + 128],
                                       op0=ALU.mult, op1=ALU.mult,
                                       accum_out=r[:])
        rs.append(r)
    rab = gact.tile([BC, 1], F32, tag="rab")
    nc.vector.tensor_add(rab[:], rs[0][:], rs[1][:])
    rcd = gact.tile([BC, 1], F32, tag="rcd")
    nc.vector.tensor_add(rcd[:], rs[2][:], rs[3][:])
    ro = gact.tile([BC, 1], F32, tag="ro")
    nc.vector.tensor_add(ro[:], rab[:], rcd[:])
    nc.sync.dma_start(out=d_out, in_=ro[:])


def _prep_core_inputs(x, W_w, W_b, fc_w, fc_b, core, nsteps=T):
    """Host-side shard + relayout for one core."""
    xs = x[core * BC:(core + 1) * BC, :nsteps]          # [BC, t, I]
    xt = np.ascontiguousarray(xs.transpose(1, 2, 0))    # [t, I, BC]
    xt = xt.reshape(nsteps, KX, 128, BC).transpose(0, 2, 1, 3)
    xt = np.ascontiguousarray(xt)                       # [t, 128, KX, BC]

    # gate reorder (f, i, o, c_hat) -> (f, i, c_hat, o)
    perm = np.concatenate([np.arange(0, 1024), np.arange(1536, 2048),
                           np.arange(1024, 1536)])
    WT = W_w.T[:, perm]                                 # [768, 2048]
    wt = np.ascontiguousarray(WT.reshape(NK, 128, 4 * H))
    # bias cols per ph: [f 256 | i 256 | c 256 | o 256] -> [1, 2, 1024]
    wb_re = np.ascontiguousarray(
        W_b[perm].reshape(4, 2, 256).transpose(1, 0, 2).reshape(1, 2, 1024))

    ones_row = np.ones((1, BC), dtype=np.float32)
    eye2 = np.eye(BC, dtype=np.float32)
    fcw = np.ascontiguousarray(
        np.broadcast_to(fc_w.reshape(1, H), (BC, H)).astype(np.float32))

    import ml_dtypes
    bf = ml_dtypes.bfloat16
    return {"xT": xt.astype(bf), "W": wt.astype(bf),
            "Wb": wb_re.astype(bf), "ones_row": ones_row.astype(bf),
            "eye2": eye2.astype(bf), "fcw": fcw}


def kernel(x, W_w, W_b, fc_w, fc_b):
    x = np.asarray(x, dtype=np.float32)
    W_w = np.asarray(W_w, dtype=np.float32)
    W_b = np.asarray(W_b, dtype=np.float32)
    fc_w = np.asarray(fc_w, dtype=np.float32)
    fc_b = np.asarray(fc_b, dtype=np.float32)

    nc = _build(T)
    in_maps = [_prep_core_inputs(x, W_w, W_b, fc_w, fc_b, c)
               for c in range(NCORES)]
    res = run_bass_kernel_spmd(nc, in_maps, list(range(NCORES))).results
    return np.concatenate(
        [res[c]["out"] + np.float32(fc_b[0]) for c in range(NCORES)], axis=0)
